# revision 24
# baseline (speedup 1.0000x reference)
"""LIF (leaky integrate-and-fire) forward kernel for Trainium2, 8 NeuronCores.

Reference recurrence (per element of [B, N], serial over T):
    v_t = DECAY * (v_{t-1} * (1 - s_{t-1})) + x_t      (REST = 0)
    s_t = (v_t > THRESHOLD)

Scaled-state formulation.  With c_t a per-step compile-time scale where
c_t = c_{t-1} / DECAY (up to exact power-of-2 renormalizations), and
q_t := c_t * v_t, x'_t := c_t * x_t (host pre-scales):

    q_{t+1} = [q_t <= c_t*THR] * q_t * rho_{t+1} + x'_{t+1}

The DECAY multiply vanishes into the scale schedule; renormalization every
5 steps (exact powers of two, folded free into the mask's second scalar)
keeps q in fp16 range (|q| < 5e3).  Verified bit-accurately on the seed-0
inputs: ~2.2e3 of 23.6e6 spikes flip (rel err 9.6e-3 < 2e-2 gate).

Engine mapping (driven by measured TRN2 ISA behavior — DVE
scalar_tensor_tensor has no fast mode (1x), tensor_scalar runs 4x and
tensor_tensor 2x on all-fp16 operands; GPSIMD has no ALU; no engine can
both multiply tensors and accumulate, so the step is three DVE ops):
  * DVE per step: mask = tensor_scalar(q is_le thr_t, * rho)   [4x]
                  r    = tensor_tensor(mask * q)               [2x]
                  X   += tensor_tensor(r + X) in-place         [2x]
    where X is the DMA-prefetched x'_{t+1} tile (input loads off-chain).
  * Act per step: s_t = Sign(q_t - thr_t) -> fp8 straight to HBM (1 B/elem;
    host decodes spike = (s > 0)).  Cheaper overall than tensor-engine
    bit-packing, which kept PE ~100 us busy to save 6 MiB of DMA.
  * input prefetches ride the SP (sync) DMA queue; spike stores ride the
    GPSIMD software-DGE queue so they can never stall a prefetch.

Sharding: batch dim (128) split 16 rows/core across 8 cores; per-core,
per-step slab is a contiguous 512 KiB fp16 block viewed as [128, 2048].
"""

import numpy as np

import concourse.bacc as bacc
import concourse.mybir as mybir
from concourse.tile import TileContext
from concourse.bass_utils import run_bass_kernel_spmd

T, B, N = 32, 128, 16384
N_CORES = 8
B_SH = B // N_CORES          # 16 batch rows per core
S = B_SH * N                 # 262144 elements per core per time step
P = 128                      # SBUF partitions
F = S // P                   # 2048 free-dim elements
DECAY = 0.2
THR = 0.3

# scale schedule: c[t] = c[t-1]*5, renormalized by exact 2^-e at steps in ES
ES = {5: 11, 10: 12, 15: 12, 20: 11, 25: 12, 30: 12}
C_SCHED = [1.0]
for _t in range(1, T):
    _c = C_SCHED[-1] * 5.0
    if _t in ES:
        _c *= 2.0 ** -ES[_t]
    C_SCHED.append(_c)
RHO = {_t - 1: 2.0 ** -ES[_t] for _t in ES}       # renorm factor used at step t
THR_T = [float(np.float32(THR * c)) for c in C_SCHED]

TRACE = False                # set True (e.g. from test.py) to capture a profile

_BUILT = {}


def _build_nc():
    nc = bacc.Bacc("TRN2", debug=False, num_devices=N_CORES)
    f32 = mybir.dt.float32
    f16 = mybir.dt.float16
    f8 = mybir.dt.float8e4
    Alu = mybir.AluOpType
    Act = mybir.ActivationFunctionType

    x = nc.dram_tensor("x", [T, S], f16, kind="ExternalInput").ap()
    y = nc.dram_tensor("y", [T, S], f8, kind="ExternalOutput").ap()
    xr = x.rearrange("t (p f) -> t p f", p=P)
    yr = y.rearrange("t (p f) -> t p f", p=P)

    with TileContext(nc) as tc:
        with (
            tc.tile_pool(name="qin", bufs=12) as q_pool,
            tc.tile_pool(name="mask", bufs=3) as m_pool,
            tc.tile_pool(name="rres", bufs=3) as r_pool,
            tc.tile_pool(name="sgn", bufs=6) as s_pool,
        ):
            # issue the first input loads before anything else so compute
            # can start as soon as the constants are ready
            qt = q_pool.tile([P, F], f16)
            nc.sync.dma_start(out=qt[:], in_=xr[0])
            q1 = q_pool.tile([P, F], f16)
            nc.sync.dma_start(out=q1[:], in_=xr[1])
            negthr = {}
            for t in range(T):
                ap = nc.alloc_sbuf_tensor(f"const_negthr{t}", [P, 1], f32).ap()
                nc.gpsimd.memset(ap, -THR_T[t])
                negthr[t] = ap
            for t in range(T):
                # spikes: Sign(q - thr_t) -> fp8 {-1,0,1}; host reads (>0)
                st = s_pool.tile([P, F], f8)
                nc.scalar.activation(st[:], qt[:], Act.Sign, bias=negthr[t])
                nc.gpsimd.dma_start(out=yr[t], in_=st[:])

                if t < T - 1:
                    q_next = q1
                    if t + 2 < T:
                        q1 = q_pool.tile([P, F], f16)
                        nc.sync.dma_start(out=q1[:], in_=xr[t + 2])
                    # mask = [q <= thr_t] * rho   (4x tensor_scalar)
                    mt = m_pool.tile([P, F], f16)
                    nc.vector.tensor_scalar(
                        out=mt[:], in0=qt[:], scalar1=THR_T[t],
                        scalar2=RHO.get(t, 1.0), op0=Alu.is_le, op1=Alu.mult,
                    )
                    # r = mask * q ; X_{t+1} += r   (2x tensor_tensors)
                    rt = r_pool.tile([P, F], f16)
                    nc.vector.tensor_tensor(
                        out=rt[:], in0=mt[:], in1=qt[:], op=Alu.mult)
                    nc.vector.tensor_tensor(
                        out=q_next[:], in0=rt[:], in1=q_next[:], op=Alu.add)
                    qt = q_next
    nc.compile()
    return nc


LAST_RESULTS = None


def kernel(tx):
    global LAST_RESULTS
    tx = np.asarray(tx)
    assert tx.shape == (T, B, N) and tx.dtype == np.float32

    if "nc" not in _BUILT:
        _BUILT["nc"] = _build_nc()
    nc = _BUILT["nc"]

    xs = np.empty((T, B, N), np.float16)
    for t in range(T):
        xs[t] = (tx[t] * np.float32(C_SCHED[t])).astype(np.float16)
    in_maps = [
        {"x": np.ascontiguousarray(
            xs[:, c * B_SH:(c + 1) * B_SH, :]).reshape(T, S)}
        for c in range(N_CORES)
    ]
    res = run_bass_kernel_spmd(nc, in_maps, core_ids=list(range(N_CORES)),
                               trace=TRACE)
    LAST_RESULTS = res

    out = np.empty((T, B, N), dtype=np.float32)
    for c in range(N_CORES):
        sgn = np.asarray(res.results[c]["y"]).astype(np.float32)
        out[:, c * B_SH:(c + 1) * B_SH, :] = (sgn > 0).reshape(T, B_SH, N)
    return out
